# revision 54
# baseline (speedup 1.0000x reference)
"""Trainium2 Bass kernel for nn_MultiHeadAttention_71210557768100.

Data-parallel over batch: 16 batches -> 8 NeuronCores, 2 batches/core.
Per core, a single fused kernel (K1) computes QKV projections, per-head
softmax attention over channel tokens, the fc projection + residual, and
per-channel partial BatchNorm statistics. The host only concatenates the
8 cores' partial stats; a second tiny kernel (K2) reduces the stats
on-device and applies the BatchNorm scale/shift.

Perf strategy vs v1:
  * QKV projections and the fc matmul run in fp8(e4m3) with
    perf_mode=DoubleRow (2 k-tiles per pass, ~2x PE throughput).
    Weights are pre-scaled x64 (fc: x128) on the host so fp8 values sit
    in the normal range; scales are undone in the PSUM->SBUF copies.
  * V is projected activation-stationary producing V[t, p] directly, so
    the per-head PE transposes of V^T disappear from the attention phase.
  * fc output (+ residual via identity*2048 matmul) is kept scaled by
    2048 end-to-end: BatchNorm is scale-invariant, so K2 is unchanged.
  * All fc weights are preloaded into SBUF during the attention phase:
    phase C runs back-to-back matmuls with no DMA bubbles (keeps the PE
    HAM clock-gate warm).
  * Intermediate out and K2 I/O are fp16 with 1 MiB contiguous DMAs.

Layouts (per core; t = 512 local tokens, f = 4096, p = 2048):
  Q^T,K^T [p,t] <- lhsT=W8[f,p-chunk] (fp8 DR), rhs=x8^T[f,t] (fp8)
  V[t,p]       <- lhsT=x8^T[f,t-chunk] (fp8 DR), rhs=Wv8^T[f,p]
  S^T[d,c]     <- lhsT=K^T[e,d], rhs=Q^T[e,c]   (1/TEMP folded into Wq)
  softmax: exp (|S| small), colsum via (1/16)-matmul, r=16/sum via
           exp(-ln(cs)) LUTs, broadcast via K=1 matmul
  A^T=16*O^T [p,t] fp8 <- lhsT=V[d,e-chunk], rhs=exp(S^T); r folded
  FC[t,f]      <- lhsT=A^T[p,t-chunk] (fp8 DR), rhs=Wfc8^T[p,f]
                  + ident(2048) @ xv  (residual, fp16)
"""

import os
import sys
import types
from contextlib import ExitStack

import numpy as np

if os.path.isdir("/opt/trn_rl_repo") and "/opt/trn_rl_repo" not in sys.path:
    sys.path.insert(0, "/opt/trn_rl_repo")

import concourse.bass as bass
import concourse.tile as tile
from concourse import mybir
from concourse.bass_utils import run_bass_kernel_spmd

F32 = mybir.dt.float32
F32R = mybir.dt.float32r
BF16 = mybir.dt.bfloat16
FP16 = mybir.dt.float16
FP8 = mybir.dt.float8e4
AX = mybir.AxisListType
ALU = mybir.AluOpType
ACTF = mybir.ActivationFunctionType
DR = mybir.MatmulPerfMode.DoubleRow

# Problem shapes (hardcoded per contract)
B, C, H, W, D = 16, 256, 16, 16, 16
F = H * W * D            # 4096 feature dim (in_pixels)
NH, LD = 8, 256
P = NH * LD              # 2048 projection dim
TEMP = float(np.sqrt(F))
EPS = 1e-5
NCORES = 8
BL = B // NCORES         # 2 local batches
T = BL * C               # 512 local tokens
FT = F // 128            # 32 f-tiles
PC = P // 128            # 16 p-chunks
NTC = T // 128           # 4 t-chunks
NFC = F // 512           # 8 f-chunks (512 wide)
NKP = FT // 2            # 16 f-tile pairs (DoubleRow)
NTOT = B * F             # BN reduction count per channel


# ---------------------------------------------------------------------------
# Workaround: this walrus build accepts at most one sync wait per Drain.
# TileContext's tail drain carries every outstanding sem wait on one SP
# drain; split them one wait per drain.
def _patch_tile_drain():
    import bass_rust as _br

    if getattr(tile.TileContext, "_drain_split_patched", False):
        return

    def _split_drain_and_barrier(self, tick_clock, wait_clock):
        nc = self.nc
        drain_inst = nc.sync.drain()
        wait_clock.add_sem_waits(
            drain_inst.ins, tile.ScopedClock({None: tick_clock.global_clock})
        )
        si = drain_inst.ins.sync_info
        waits = list(si.on_wait) if si is not None else []
        if len(waits) > 1:
            si.on_wait = waits[:1]
            for w in waits[1:]:
                d2 = nc.sync.drain()
                d2.ins.sync_info = _br.SyncInfo(on_wait=[w], on_update=[])
        nc.all_engine_barrier()
        assert self.sems is not None
        popped = nc._tile_sem_poison_stack.pop()
        assert popped is self._sem_poison
        nc.clear_and_free_semaphores(list(self.sems.allocated().values()))
        nc.all_engine_barrier()

    tile.TileContext._drain_and_barrier = _split_drain_and_barrier
    tile.TileContext._drain_split_patched = True


_patch_tile_drain()


# Workaround (general form): this walrus build accepts at most ONE sync
# wait per instruction. Post-process the serialized BIR: any instruction
# carrying N>1 waits keeps its last wait; the other N-1 move onto NoOp
# instructions inserted just before it on the same engine (same-engine
# program order makes this equivalent).
def _split_waits_in_json(raw: bytes) -> bytes:
    import json

    data = json.loads(raw)
    counter = [0]
    changed = False
    for fn in data.get("functions", []):
        for blk in fn.get("blocks", []):
            insts = blk.get("instructions", [])
            out = []
            for inst in insts:
                si = inst.get("sync_info")
                waits = si.get("on_wait") if si else None
                if waits and len(waits) > 1:
                    changed = True
                    eng = inst.get("engine")
                    for w in waits[:-1]:
                        counter[0] += 1
                        out.append(
                            {
                                "engine": eng,
                                "ins": [],
                                "name": f"I-wsplit-{counter[0]}",
                                "opcode": "NoOp",
                                "outs": [],
                                "sync_info": {"on_wait": [w], "on_update": []},
                            }
                        )
                    si["on_wait"] = waits[-1:]
                out.append(inst)
            if changed:
                blk["instructions"] = out
    if not changed:
        return raw
    return json.dumps(data).encode()


def _patch_wait_split():
    if getattr(bass.Bass, "_wait_split_patched", False):
        return
    orig = bass.Bass.to_json_bytes

    def to_json_bytes(self):
        return _split_waits_in_json(orig(self))

    bass.Bass.to_json_bytes = to_json_bytes
    bass.Bass._wait_split_patched = True


_patch_wait_split()


# NTFF profiling hook (for trace=True timing): register the ctypes-based
# hook if the antenv.axon_hooks module is missing in this image.
def _ensure_ntff_hook():
    try:
        import antenv.axon_hooks  # noqa: F401

        return
    except ImportError:
        pass
    try:
        from trn_agent_boot.trn_boot import _ntff_profile_via_ctypes

        hook = _ntff_profile_via_ctypes("/opt/axon/libaxon_pjrt.so")
    except Exception:
        hook = None
    mod = types.ModuleType("antenv.axon_hooks")
    mod.get_axon_ntff_profile_hook = lambda: hook
    mod.set_axon_ntff_profile_hook = lambda h: None
    sys.modules["antenv.axon_hooks"] = mod


# ---------------------------------------------------------------------------
def build_k1() -> bass.Bass:
    nc = bass.Bass("TRN2", target_bir_lowering=False, debug=False, num_devices=NCORES)

    xq8 = nc.dram_tensor("xq8", [128, FT, T], FP8, kind="ExternalInput")
    xk8 = nc.dram_tensor("xk8", [128, FT, T], FP8, kind="ExternalInput")
    xv8 = nc.dram_tensor("xv8", [128, FT, T], FP8, kind="ExternalInput")
    wq8 = nc.dram_tensor("wq8", [PC, 128, FT, 128], FP8, kind="ExternalInput")
    wk8 = nc.dram_tensor("wk8", [PC, 128, FT, 128], FP8, kind="ExternalInput")
    wv8 = nc.dram_tensor("wv8", [NKP, 128, 2, P], FP8, kind="ExternalInput")
    wfc8 = nc.dram_tensor("wfc8", [NFC, 128, 2, F], FP8, kind="ExternalInput")
    xv_res = nc.dram_tensor("xv_res", [NTC, 128, F], FP16, kind="ExternalInput")
    ones_c = nc.dram_tensor("ones_c", [128, 1], BF16, kind="ExternalInput")
    ones_r = nc.dram_tensor("ones_r", [1, 128], BF16, kind="ExternalInput")
    out_blk = nc.dram_tensor("out_blk", [NTC, 128, F], FP16, kind="ExternalOutput")
    stats = nc.dram_tensor("stats", [128, 4], F32, kind="ExternalOutput")

    with tile.TileContext(nc) as tc, ExitStack() as ctx:
        singles = ctx.enter_context(tc.tile_pool(name="singles", bufs=1))
        ones_col = singles.tile([128, 1], BF16)
        nc.sync.dma_start(out=ones_col, in_=ones_c[:, :])
        one_row = singles.tile([1, 128], BF16)
        nc.sync.dma_start(out=one_row, in_=ones_r[:, :])
        sums_buf = singles.tile([128, NTC, NFC], F32)
        sqs_buf = singles.tile([128, NTC, NFC], F32)
        tmp4 = singles.tile([128, 4], F32)
        tmp4b = singles.tile([128, 4], F32)
        stats_sb = singles.tile([128, 4], F32)

        qkv_pool = ctx.enter_context(tc.tile_pool(name="qkv", bufs=1))
        QT = qkv_pool.tile([128, PC, T], BF16, tag="QT")
        KT = qkv_pool.tile([128, PC, T], BF16, tag="KT")
        Vtp = qkv_pool.tile([128, NTC, P], BF16, tag="Vtp")

        at_pool = ctx.enter_context(tc.tile_pool(name="at", bufs=1))
        AT = at_pool.tile([128, PC, T], FP8)

        # PE warmup: dummy matmuls during the initial DMA wait so the HAM
        # clock-gate lifts to 2.4 GHz by the time real work arrives.  The
        # warmup tile lives in the persistent pool: a scoped pool here would
        # hand its SBUF to the xq tiles, whose DMA would then wait for the
        # warmup matmuls to finish (WAR) — exactly the stall being avoided.
        wup = singles.tile([128, 128], BF16)
        nc.vector.memset(wup, 0.0)
        with tc.tile_pool(name="wupps", bufs=1, space="PSUM") as wup_ps:
            wps = wup_ps.tile([128, 128], F32)
            for _ in range(12):
                nc.tensor.matmul(wps, lhsT=(wup), rhs=(wup), start=True, stop=True)

        # ---- Phase A ----
        wfc_tiles = None
        with (
            tc.tile_pool(name="xt", bufs=5) as xt_pool,
            tc.tile_pool(name="wv", bufs=16) as wv_pool,
        ):
            # A1: Q^T, K^T (weight-stationary fp8 DoubleRow).  Wv^T tiles
            # stream on the scalar queue during K so they don't steal HBM
            # bandwidth from the startup-critical wq tiles.
            wv_tiles = []
            xts_v = None
            with (
                tc.tile_pool(name="wp", bufs=3) as w_pool,
                tc.tile_pool(name="pjps", bufs=2, space="PSUM") as pj_ps,
            ):
                for xi, (xT_dram, w_dram, OUT, scl) in enumerate((
                    (xq8, wq8, QT, 2.0**-12),
                    (xk8, wk8, KT, 2.0**-6),
                )):
                    xts = []
                    for q4 in range(4):
                        xq_t = xt_pool.tile([128, 8, T], FP8, tag="xt")
                        nc.sync.dma_start(
                            out=xq_t, in_=xT_dram[:, q4 * 8 : (q4 + 1) * 8, :]
                        )
                        xts.append(xq_t)
                    for pc in range(PC):
                        w = w_pool.tile([128, FT, 128], FP8, tag="w")
                        nc.gpsimd.dma_start(out=w, in_=w_dram[pc])
                        # Wv^T preload rides the same gpsimd DMA ring, behind
                        # wq/wk: ring FIFO + the w-pool's buffer pacing push
                        # the 8 MiB wv stream off the startup-critical path
                        # (engine program order does NOT order DMA dispatches,
                        # so a different queue would start transferring at t=0)
                        if len(wv_tiles) < NKP and (xi == 1 or pc >= 3):
                            wvt = wv_pool.tile([128, 2, P], FP8, tag="wv")
                            nc.gpsimd.dma_start(out=wvt, in_=wv8[len(wv_tiles)])
                            wv_tiles.append(wvt)
                        ps = pj_ps.tile([128, T], F32, tag="pj")
                        for jj in range(NKP):
                            m = (2 * jj) % 8
                            nc.tensor.matmul(
                                ps,
                                lhsT=(w[:, 2 * jj : 2 * jj + 2, :]),
                                rhs=(xts[jj // 4][:, m : m + 2, :]),
                                start=(jj == 0),
                                stop=(jj == NKP - 1),
                                perf_mode=DR,
                            )
                        nc.vector.tensor_scalar_mul(OUT[:, pc, :], ps, scl)

                # prefetch xv tiles while K finishes
                xts_v = []
                for q4 in range(4):
                    xv_t = xt_pool.tile([128, 8, T], FP8, tag="xt")
                    nc.sync.dma_start(
                        out=xv_t, in_=xv8[:, q4 * 8 : (q4 + 1) * 8, :]
                    )
                    xts_v.append(xv_t)

            # A2: V[t, p] (activation-stationary fp8 DoubleRow)
            with tc.tile_pool(name="vps", bufs=2, space="PSUM") as v_ps:
                for tcc in range(NTC):
                    vps = v_ps.tile([128, 4, 512], F32, tag="vps")
                    for kp in range(NKP):
                        m = (2 * kp) % 8
                        for pq in range(4):
                            nc.tensor.matmul(
                                vps[:, pq, :],
                                lhsT=(
                                    xts_v[kp // 4][:, m : m + 2, tcc * 128 : (tcc + 1) * 128]
                                ),
                                rhs=(wv_tiles[kp][:, :, pq * 512 : (pq + 1) * 512]),
                                start=(kp == 0),
                                stop=(kp == NKP - 1),
                                perf_mode=DR,
                            )
                    for pq in range(4):
                        # split psum->SBUF copies across DVE and ACT so the
                        # trailing copies (which gate phase B's psum banks)
                        # drain twice as fast
                        if pq % 2 == 0:
                            nc.vector.tensor_scalar_mul(
                                Vtp[:, tcc, pq * 512 : (pq + 1) * 512],
                                vps[:, pq, :],
                                2.0**-6,
                            )
                        else:
                            nc.scalar.mul(
                                Vtp[:, tcc, pq * 512 : (pq + 1) * 512],
                                vps[:, pq, :],
                                2.0**-6,
                            )

        # ---- Phase B: attention -> AT = 16*O^T [p, t] fp8 ----
        # fc weights preload on the idle sync queue (SBUF freed by phase A);
        # the phase-C residual tiles prefetch on gpsimd, also idle during B
        wfc_pool = ctx.enter_context(tc.tile_pool(name="wfc", bufs=8))
        wfc_tiles = []
        for j in range(NFC):
            wt = wfc_pool.tile([128, 2, F], FP8, tag="wfc")
            nc.sync.dma_start(out=wt, in_=wfc8[j])
            wfc_tiles.append(wt)
        xvr_pool = ctx.enter_context(tc.tile_pool(name="xvr", bufs=4))
        xv_tiles = []
        for tcc in range(NTC):
            xvt = xvr_pool.tile([128, F], FP16, tag="xv")
            nc.gpsimd.dma_start(out=xvt, in_=xv_res[tcc])
            xv_tiles.append(xvt)

        with (
            tc.tile_pool(name="asb", bufs=6) as asb,
            tc.tile_pool(name="stps", bufs=2, space="PSUM") as st_ps,
            tc.tile_pool(name="otps", bufs=1, space="PSUM") as ot_ps,
            tc.tile_pool(name="csps", bufs=1, space="PSUM") as cs_ps,
            tc.tile_pool(name="rbps", bufs=1, space="PSUM") as rb_ps,
        ):
            # two heads per iteration: wider ACT/PE ops, half the chain count
            iters = [(b, n) for b in range(BL) for n in range(0, NH, 2)]

            def emit_s(b, n):
                # S^T[d, (h2,c)] = sum_e K[d,e] Q[c,e] (head-transposed)
                t0 = b * C
                st = st_ps.tile([128, 2, 2, 256], F32, tag="st")
                for dc in range(2):
                    for h2 in range(2):
                        for et in range(2):
                            nc.tensor.matmul(
                                st[:, dc, h2, :],
                                lhsT=(
                                    KT[:, 2 * (n + h2) + et, t0 + dc * 128 : t0 + (dc + 1) * 128]
                                ),
                                rhs=(QT[:, 2 * (n + h2) + et, t0 : t0 + 256]),
                                start=(et == 0),
                                stop=(et == 1),
                            )
                return st

            st_next = emit_s(*iters[0])
            for it, (b, n) in enumerate(iters):
                    t0 = b * C
                    st = st_next
                    et_sb = asb.tile([128, 2, 2, 256], BF16, tag="et")
                    for dc in range(2):
                        nc.scalar.activation(
                            out=et_sb[:, dc], in_=st[:, dc], func=ACTF.Exp
                        )
                    # software pipeline: the next iteration's S matmuls issue
                    # before this iteration's cs/ot so the PE has independent
                    # work while the ACT exp chain runs (PE is in-order).
                    if it + 1 < len(iters):
                        st_next = emit_s(*iters[it + 1])
                    # column sums over d (partition axis) via (1/16)-matmul:
                    # cs = colsum/16, so r = exp(-ln(cs)) = 16/colsum and the
                    # fp8 A^T picks up the x16 fp8-friendly scale for free.
                    cs = cs_ps.tile([1, 512], F32, tag="cs")
                    for dc in range(2):
                        nc.tensor.matmul(
                            cs,
                            lhsT=(ones_col),
                            rhs=(et_sb[:, dc]),
                            start=(dc == 0),
                            stop=(dc == 1),
                        )
                    ln_sb = asb.tile([1, 512], F32, tag="lnsb")
                    nc.scalar.activation(out=ln_sb, in_=cs, func=ACTF.Ln)
                    r_sb = asb.tile([1, 512], BF16, tag="rsb")
                    nc.scalar.activation(out=r_sb, in_=ln_sb, func=ACTF.Exp, scale=-1.0)
                    # O^T[e, (h2,c)] = sum_d V[d,e] expS^T[d,c]  (V is [t,p])
                    ot = ot_ps.tile([128, 2, 2, 256], F32, tag="ot")
                    for h2 in range(2):
                        for ec in range(2):
                            for dc in range(2):
                                nc.tensor.matmul(
                                    ot[:, ec, h2, :],
                                    lhsT=(
                                        Vtp[
                                            :,
                                            b * 2 + dc,
                                            256 * (n + h2) + ec * 128 : 256 * (n + h2) + (ec + 1) * 128,
                                        ]
                                    ),
                                    rhs=(et_sb[:, dc, h2]),
                                    start=(dc == 0),
                                    stop=(dc == 1),
                                )
                    # broadcast r across partitions via K=1 matmul; issued
                    # after the ot matmuls so the ACT ln/exp chain hides
                    # behind PE work (the PE executes in program order)
                    rb = rb_ps.tile([128, 512], F32, tag="rb")
                    nc.tensor.matmul(rb, lhsT=(one_row), rhs=(r_sb), start=True, stop=True)
                    rb_sb = asb.tile([128, 512], F32, tag="rbs")
                    nc.vector.tensor_copy(out=rb_sb, in_=rb)
                    # AT p-chunks are laid out (ec, n) — heads adjacent — so
                    # both heads' chunks write in ONE strided DVE op per ec
                    # (the host permutes Wfc rows to match)
                    for ec in range(2):
                        nc.vector.tensor_mul(
                            out=AT[:, ec * 8 + n : ec * 8 + n + 2, t0 : t0 + 256],
                            in0=ot[:, ec],
                            in1=rb_sb,
                        )

        # ---- Phase C: FC + residual + BN partial stats (all x2048) ----
        with (
            tc.tile_pool(name="orow", bufs=2) as orow_pool,
            tc.tile_pool(name="sqp", bufs=2) as sq_pool,
            tc.tile_pool(name="fcps", bufs=8, space="PSUM") as fc_ps,
        ):
            for tcc in range(NTC):
                xvt = xv_tiles[tcc]
                orow = orow_pool.tile([128, F], FP16, tag="orow")
                for fc_ in range(NFC):
                    ps = fc_ps.tile([128, 512], F32, tag="fc")
                    for j in range(NFC):
                        nc.tensor.matmul(
                            ps,
                            lhsT=(AT[:, 2 * j : 2 * j + 2, tcc * 128 : (tcc + 1) * 128]),
                            rhs=(wfc_tiles[j][:, :, fc_ * 512 : (fc_ + 1) * 512]),
                            start=(j == 0),
                            stop=(j == NFC - 1),
                            perf_mode=DR,
                        )
                    ores = orow[:, fc_ * 512 : (fc_ + 1) * 512]
                    # residual: out = ps + 2048*xv (xv_res pre-scaled on host)
                    nc.vector.tensor_add(
                        ores, ps, xvt[:, fc_ * 512 : (fc_ + 1) * 512]
                    )
                    nc.vector.reduce_sum(
                        out=sums_buf[:, tcc, fc_ : fc_ + 1], in_=ores, axis=AX.X
                    )
                    sqt = sq_pool.tile([128, 512], F32, tag="sq")
                    nc.scalar.activation(
                        out=sqt,
                        in_=ores,
                        func=ACTF.Square,
                        accum_out=sqs_buf[:, tcc, fc_ : fc_ + 1],
                    )
                    if fc_ % 2 == 1:
                        qlo, qhi = (fc_ - 1) * 512, (fc_ + 1) * 512
                        nc.sync.dma_start(
                            out=out_blk[tcc][:, qlo:qhi], in_=orow[:, qlo:qhi]
                        )

            # fold partial sums: per t-chunk reduce over f-chunks, then add
            # the two batches per channel-half
            for tcc in range(NTC):
                nc.vector.reduce_sum(out=tmp4[:, tcc : tcc + 1], in_=sums_buf[:, tcc, :], axis=AX.X)
                nc.vector.reduce_sum(out=tmp4b[:, tcc : tcc + 1], in_=sqs_buf[:, tcc, :], axis=AX.X)
            nc.vector.tensor_add(stats_sb[:, 0:1], tmp4[:, 0:1], tmp4[:, 2:3])
            nc.vector.tensor_add(stats_sb[:, 1:2], tmp4[:, 1:2], tmp4[:, 3:4])
            nc.vector.tensor_add(stats_sb[:, 2:3], tmp4b[:, 0:1], tmp4b[:, 2:3])
            nc.vector.tensor_add(stats_sb[:, 3:4], tmp4b[:, 1:2], tmp4b[:, 3:4])
            nc.sync.dma_start(out=stats[:, :], in_=stats_sb)

    return nc


def build_k2() -> bass.Bass:
    nc = bass.Bass("TRN2", target_bir_lowering=False, debug=False, num_devices=NCORES)

    x_blk = nc.dram_tensor("x_blk", [NTC, 128, F], FP16, kind="ExternalInput")
    stats_all = nc.dram_tensor("stats_all", [4, 128, NCORES], F32, kind="ExternalInput")
    gamma2 = nc.dram_tensor("gamma2", [128, 2], F32, kind="ExternalInput")
    beta2 = nc.dram_tensor("beta2", [128, 2], F32, kind="ExternalInput")
    y_blk = nc.dram_tensor("y_blk", [NTC, 128, F], FP16, kind="ExternalOutput")

    with tile.TileContext(nc) as tc, ExitStack() as ctx:
        singles = ctx.enter_context(tc.tile_pool(name="singles", bufs=1))
        xpool = ctx.enter_context(tc.tile_pool(name="xin", bufs=4))
        ypool = ctx.enter_context(tc.tile_pool(name="yout", bufs=2))
        # x loads are the critical path: issue them first on the sync queue
        xin = []
        for tcc in range(NTC):
            t = xpool.tile([128, F], FP16, tag="in")
            nc.sync.dma_start(out=t, in_=x_blk[tcc])
            xin.append(t)
        # stats/constants on the gpsimd queue (host pre-transposed, so these
        # are few large-ish descriptors, not thousands of 4B packets)
        st_sb = singles.tile([128, 4, NCORES], F32)
        for j in range(4):
            nc.gpsimd.dma_start(out=st_sb[:, j, :], in_=stats_all[j])
        gam = singles.tile([128, 2], F32)
        nc.gpsimd.dma_start(out=gam, in_=gamma2[:, :])
        bet = singles.tile([128, 2], F32)
        nc.gpsimd.dma_start(out=bet, in_=beta2[:, :])
        eps_sb = singles.tile([128, 1], F32)
        nc.vector.memset(eps_sb, EPS)

        mean_sb = singles.tile([128, 2], F32)
        msq_sb = singles.tile([128, 2], F32)
        m2_sb = singles.tile([128, 2], F32)
        var_sb = singles.tile([128, 2], F32)
        std_sb = singles.tile([128, 2], F32)
        rstd_sb = singles.tile([128, 2], F32)
        scale_sb = singles.tile([128, 2], F32)
        shf_sb = singles.tile([128, 2], F32)
        tmp_sb = singles.tile([128, 2], F32)

        tot = singles.tile([128, 4], F32)
        nc.vector.reduce_sum(out=tot, in_=st_sb, axis=AX.X)
        inv_n = 1.0 / float(NTOT)
        nc.vector.tensor_scalar_mul(mean_sb, tot[:, 0:2], inv_n)
        nc.vector.tensor_scalar_mul(msq_sb, tot[:, 2:4], inv_n)
        nc.vector.tensor_mul(m2_sb, mean_sb, mean_sb)
        nc.vector.tensor_sub(var_sb, msq_sb, m2_sb)
        nc.scalar.activation(out=std_sb, in_=var_sb, func=ACTF.Sqrt, bias=eps_sb)
        nc.vector.reciprocal(out=rstd_sb, in_=std_sb)
        nc.vector.tensor_mul(scale_sb, gam, rstd_sb)
        nc.vector.tensor_mul(tmp_sb, mean_sb, scale_sb)
        nc.vector.tensor_sub(shf_sb, bet, tmp_sb)

        for tcc in range(NTC):
            j = tcc % 2
            y = ypool.tile([128, F], FP16, tag="y")
            nc.vector.tensor_scalar(
                out=y,
                in0=xin[tcc],
                scalar1=scale_sb[:, j : j + 1],
                scalar2=shf_sb[:, j : j + 1],
                op0=ALU.mult,
                op1=ALU.add,
            )
            nc.sync.dma_start(out=y_blk[tcc], in_=y)

    return nc


# ---------------------------------------------------------------------------
# Host-side layout prep
def _prep_weights(Wq, Wk, Wv, Wfc):
    import ml_dtypes

    f8 = ml_dtypes.float8_e4m3

    def blk_w(Wt):  # [P, F] -> [PC, 128, FT, 128] (f-major blocked, p-chunked)
        return np.ascontiguousarray(
            Wt.T.reshape(FT, 128, PC, 128).transpose(2, 1, 0, 3).astype(f8)
        )

    # x64 lifts fp8 values to ~unit std; undone in the PSUM->SBUF copies
    # (Q's 2^-12 copy scale = 1/64 fp8-undo * 1/TEMP softmax temperature)
    wq = blk_w(np.asarray(Wq, np.float32) * 64.0)
    wk = blk_w(np.asarray(Wk, np.float32) * 64.0)
    # Wv^T [F, P] -> [NKP, 128, 2, P] (k-tile pairs for DoubleRow rhs)
    wv = np.ascontiguousarray(
        (np.asarray(Wv, np.float32).T * 64.0)
        .reshape(NKP, 2, 128, P)
        .transpose(0, 2, 1, 3)
        .astype(f8)
    )
    # Wfc [F, P] -> Wfc^T [P, F], rows permuted from (n, ec) to (ec, n) order
    # to match AT's p-chunk layout, -> [NFC, 128, 2, F] (p-tile pairs), x128
    wfcT = (np.asarray(Wfc, np.float32).T * 128.0).reshape(NH, 2, 128, F)
    wfcT = wfcT.transpose(1, 0, 2, 3).reshape(P, F)
    wfc = np.ascontiguousarray(
        wfcT.reshape(NFC, 2, 128, F).transpose(0, 2, 1, 3).astype(f8)
    )
    return wq, wk, wv, wfc


_BUILT = {}


def _get_built(name):
    if name not in _BUILT:
        _BUILT[name] = build_k1() if name == "k1" else build_k2()
    return _BUILT[name]


def run_full(v, k, q, Wq, Wk, Wv, Wfc, gamma, beta, trace=False):
    """Returns (y [16,256,16,16,16] fp32, exec_ns_k1, exec_ns_k2)."""
    import ml_dtypes

    f8 = ml_dtypes.float8_e4m3
    if trace:
        _ensure_ntff_hook()
    q3 = np.asarray(q, np.float32).reshape(B, C, F)
    k3 = np.asarray(k, np.float32).reshape(B, C, F)
    v3 = np.asarray(v, np.float32).reshape(B, C, F)
    wq, wk, wv, wfc = _prep_weights(Wq, Wk, Wv, Wfc)

    def xpm(x):  # [T, F] -> partition-major x^T blocks [128, FT, T]
        return np.ascontiguousarray(x.T.reshape(FT, 128, T).transpose(1, 0, 2).astype(f8))

    in_maps = []
    for ci in range(NCORES):
        b0 = ci * BL
        xq = q3[b0 : b0 + BL].reshape(T, F)
        xk = k3[b0 : b0 + BL].reshape(T, F)
        xv = v3[b0 : b0 + BL].reshape(T, F)
        in_maps.append(
            {
                "xq8": xpm(xq),
                "xk8": xpm(xk),
                "xv8": xpm(xv),
                "wq8": wq,
                "wk8": wk,
                "wv8": wv,
                "wfc8": wfc,
                "xv_res": np.ascontiguousarray(
                    (xv * 2048.0).reshape(NTC, 128, F).astype(np.float16)
                ),
                "ones_c": np.full((128, 1), 1.0 / 16.0, ml_dtypes.bfloat16),
                "ones_r": np.ones((1, 128), ml_dtypes.bfloat16),
            }
        )

    nc1 = _get_built("k1")
    res1 = run_bass_kernel_spmd(nc1, in_maps, core_ids=list(range(NCORES)), trace=trace)
    t1 = res1.exec_time_ns

    # per-core stats come back [128, 4]; K2 wants [4, 128, NCORES]
    stats_all = np.ascontiguousarray(
        np.stack([res1.results[ci]["stats"] for ci in range(NCORES)]).transpose(2, 1, 0)
    )
    gamma2 = np.ascontiguousarray(np.asarray(gamma, np.float32).reshape(2, 128).T)
    beta2 = np.ascontiguousarray(np.asarray(beta, np.float32).reshape(2, 128).T)

    in_maps2 = [
        {
            "x_blk": res1.results[ci]["out_blk"],
            "stats_all": stats_all,
            "gamma2": gamma2,
            "beta2": beta2,
        }
        for ci in range(NCORES)
    ]
    nc2 = _get_built("k2")
    res2 = run_bass_kernel_spmd(nc2, in_maps2, core_ids=list(range(NCORES)), trace=trace)
    t2 = res2.exec_time_ns

    y = np.empty((B, C, F), np.float32)
    for ci in range(NCORES):
        y[ci * BL : (ci + 1) * BL] = (
            res2.results[ci]["y_blk"].astype(np.float32).reshape(T, F).reshape(BL, C, F)
        )
    return y.reshape(B, C, H, W, D), t1, t2


def kernel(**inputs) -> np.ndarray:
    y, _, _ = run_full(**inputs)
    return y


# revision 56
# speedup vs baseline: 1.0346x; 1.0346x over previous
"""Trainium2 Bass kernel for nn_MultiHeadAttention_71210557768100.

Data-parallel over batch: 16 batches -> 8 NeuronCores, 2 batches/core.
Per core, a single fused kernel (K1) computes QKV projections, per-head
softmax attention over channel tokens, the fc projection + residual, and
per-channel partial BatchNorm statistics. The host only concatenates the
8 cores' partial stats; a second tiny kernel (K2) reduces the stats
on-device and applies the BatchNorm scale/shift.

Perf strategy vs v1:
  * QKV projections and the fc matmul run in fp8(e4m3) with
    perf_mode=DoubleRow (2 k-tiles per pass, ~2x PE throughput).
    Weights are pre-scaled x64 (fc: x128) on the host so fp8 values sit
    in the normal range; scales are undone in the PSUM->SBUF copies.
  * V is projected activation-stationary producing V[t, p] directly, so
    the per-head PE transposes of V^T disappear from the attention phase.
  * fc output (+ residual added on the DVE from host-prescaled 2048*xv)
    is kept scaled by 2048 end-to-end: BatchNorm is scale-invariant, so
    K2's math is unchanged by the scaling.
  * All fc weights are preloaded into SBUF during the attention phase:
    phase C runs back-to-back matmuls with no DMA bubbles (keeps the PE
    HAM clock-gate warm).
  * Intermediate out and K2 I/O are fp16 with 1 MiB contiguous DMAs.

Layouts (per core; t = 512 local tokens, f = 4096, p = 2048):
  Q^T,K^T [p,t] <- lhsT=W8[f,p-chunk] (fp8 DR), rhs=x8^T[f,t] (fp8)
  V[t,p]       <- lhsT=x8^T[f,t-chunk] (fp8 DR), rhs=Wv8^T[f,p]
  S^T[d,c]     <- lhsT=K^T[e,d], rhs=Q^T[e,c]   (1/TEMP folded into Wq)
  softmax: exp (|S| small), colsum via (1/16)-matmul, r=16/sum via
           exp(-ln(cs)) LUTs, broadcast via K=1 matmul
  A^T=16*O^T [p,t] fp8 <- lhsT=V[d,e-chunk], rhs=exp(S^T); r folded
  FC[t,f]      <- lhsT=A^T[p,t-chunk] (fp8 DR), rhs=Wfc8^T[p,f]
                  + 2048*xv residual (DVE add, fp16)
"""

import os
import sys
import types
from contextlib import ExitStack

import numpy as np

if os.path.isdir("/opt/trn_rl_repo") and "/opt/trn_rl_repo" not in sys.path:
    sys.path.insert(0, "/opt/trn_rl_repo")

import concourse.bass as bass
import concourse.tile as tile
from concourse import mybir
from concourse.bass_utils import run_bass_kernel_spmd

F32 = mybir.dt.float32
F32R = mybir.dt.float32r
BF16 = mybir.dt.bfloat16
FP16 = mybir.dt.float16
FP8 = mybir.dt.float8e4
AX = mybir.AxisListType
ALU = mybir.AluOpType
ACTF = mybir.ActivationFunctionType
DR = mybir.MatmulPerfMode.DoubleRow

# Problem shapes (hardcoded per contract)
B, C, H, W, D = 16, 256, 16, 16, 16
F = H * W * D            # 4096 feature dim (in_pixels)
NH, LD = 8, 256
P = NH * LD              # 2048 projection dim
TEMP = float(np.sqrt(F))
EPS = 1e-5
NCORES = 8
BL = B // NCORES         # 2 local batches
T = BL * C               # 512 local tokens
FT = F // 128            # 32 f-tiles
PC = P // 128            # 16 p-chunks
NTC = T // 128           # 4 t-chunks
NFC = F // 512           # 8 f-chunks (512 wide)
NKP = FT // 2            # 16 f-tile pairs (DoubleRow)
NTOT = B * F             # BN reduction count per channel


# ---------------------------------------------------------------------------
# Workaround: this walrus build accepts at most one sync wait per Drain.
# TileContext's tail drain carries every outstanding sem wait on one SP
# drain; split them one wait per drain.
def _patch_tile_drain():
    import bass_rust as _br

    if getattr(tile.TileContext, "_drain_split_patched", False):
        return

    def _split_drain_and_barrier(self, tick_clock, wait_clock):
        nc = self.nc
        drain_inst = nc.sync.drain()
        wait_clock.add_sem_waits(
            drain_inst.ins, tile.ScopedClock({None: tick_clock.global_clock})
        )
        si = drain_inst.ins.sync_info
        waits = list(si.on_wait) if si is not None else []
        if len(waits) > 1:
            si.on_wait = waits[:1]
            for w in waits[1:]:
                d2 = nc.sync.drain()
                d2.ins.sync_info = _br.SyncInfo(on_wait=[w], on_update=[])
        nc.all_engine_barrier()
        assert self.sems is not None
        popped = nc._tile_sem_poison_stack.pop()
        assert popped is self._sem_poison
        nc.clear_and_free_semaphores(list(self.sems.allocated().values()))
        nc.all_engine_barrier()

    tile.TileContext._drain_and_barrier = _split_drain_and_barrier
    tile.TileContext._drain_split_patched = True


_patch_tile_drain()


# Workaround (general form): this walrus build accepts at most ONE sync
# wait per instruction. Post-process the serialized BIR: any instruction
# carrying N>1 waits keeps its last wait; the other N-1 move onto NoOp
# instructions inserted just before it on the same engine (same-engine
# program order makes this equivalent).
def _split_waits_in_json(raw: bytes) -> bytes:
    import json

    data = json.loads(raw)
    counter = [0]
    changed = False
    for fn in data.get("functions", []):
        for blk in fn.get("blocks", []):
            insts = blk.get("instructions", [])
            out = []
            for inst in insts:
                si = inst.get("sync_info")
                waits = si.get("on_wait") if si else None
                if waits and len(waits) > 1:
                    changed = True
                    eng = inst.get("engine")
                    for w in waits[:-1]:
                        counter[0] += 1
                        out.append(
                            {
                                "engine": eng,
                                "ins": [],
                                "name": f"I-wsplit-{counter[0]}",
                                "opcode": "NoOp",
                                "outs": [],
                                "sync_info": {"on_wait": [w], "on_update": []},
                            }
                        )
                    si["on_wait"] = waits[-1:]
                out.append(inst)
            if changed:
                blk["instructions"] = out
    if not changed:
        return raw
    return json.dumps(data).encode()


def _patch_wait_split():
    if getattr(bass.Bass, "_wait_split_patched", False):
        return
    orig = bass.Bass.to_json_bytes

    def to_json_bytes(self):
        return _split_waits_in_json(orig(self))

    bass.Bass.to_json_bytes = to_json_bytes
    bass.Bass._wait_split_patched = True


_patch_wait_split()


# NTFF profiling hook (for trace=True timing): register the ctypes-based
# hook if the antenv.axon_hooks module is missing in this image.
def _ensure_ntff_hook():
    try:
        import antenv.axon_hooks  # noqa: F401

        return
    except ImportError:
        pass
    try:
        from trn_agent_boot.trn_boot import _ntff_profile_via_ctypes

        hook = _ntff_profile_via_ctypes("/opt/axon/libaxon_pjrt.so")
    except Exception:
        hook = None
    mod = types.ModuleType("antenv.axon_hooks")
    mod.get_axon_ntff_profile_hook = lambda: hook
    mod.set_axon_ntff_profile_hook = lambda h: None
    sys.modules["antenv.axon_hooks"] = mod


# ---------------------------------------------------------------------------
def build_k1() -> bass.Bass:
    nc = bass.Bass("TRN2", target_bir_lowering=False, debug=False, num_devices=NCORES)

    xq8 = nc.dram_tensor("xq8", [128, FT, T], FP8, kind="ExternalInput")
    xk8 = nc.dram_tensor("xk8", [128, FT, T], FP8, kind="ExternalInput")
    xv8 = nc.dram_tensor("xv8", [128, FT, T], FP8, kind="ExternalInput")
    wq8 = nc.dram_tensor("wq8", [PC, 128, FT, 128], FP8, kind="ExternalInput")
    wk8 = nc.dram_tensor("wk8", [PC, 128, FT, 128], FP8, kind="ExternalInput")
    wv8 = nc.dram_tensor("wv8", [NKP, 128, 2, P], FP8, kind="ExternalInput")
    wfc8 = nc.dram_tensor("wfc8", [NFC, 128, 2, F], FP8, kind="ExternalInput")
    xv_res = nc.dram_tensor("xv_res", [NTC, 128, F], FP16, kind="ExternalInput")
    ones_c = nc.dram_tensor("ones_c", [128, 1], BF16, kind="ExternalInput")
    ones_r = nc.dram_tensor("ones_r", [1, 128], BF16, kind="ExternalInput")
    out_blk = nc.dram_tensor("out_blk", [NTC, 128, F], FP16, kind="ExternalOutput")
    stats = nc.dram_tensor("stats", [128, 4], F32, kind="ExternalOutput")

    with tile.TileContext(nc) as tc, ExitStack() as ctx:
        singles = ctx.enter_context(tc.tile_pool(name="singles", bufs=1))
        ones_col = singles.tile([128, 1], BF16)
        nc.sync.dma_start(out=ones_col, in_=ones_c[:, :])
        one_row = singles.tile([1, 128], BF16)
        nc.sync.dma_start(out=one_row, in_=ones_r[:, :])
        sums_buf = singles.tile([128, NTC, NFC], F32)
        sqs_buf = singles.tile([128, NTC, NFC], F32)
        tmp4 = singles.tile([128, 4], F32)
        tmp4b = singles.tile([128, 4], F32)
        stats_sb = singles.tile([128, 4], F32)

        qkv_pool = ctx.enter_context(tc.tile_pool(name="qkv", bufs=1))
        QT = qkv_pool.tile([128, PC, T], BF16, tag="QT")
        KT = qkv_pool.tile([128, PC, T], BF16, tag="KT")
        Vtp = qkv_pool.tile([128, NTC, P], BF16, tag="Vtp")

        at_pool = ctx.enter_context(tc.tile_pool(name="at", bufs=1))
        AT = at_pool.tile([128, PC, T], FP8)

        # PE warmup: dummy matmuls during the initial DMA wait so the HAM
        # clock-gate lifts to 2.4 GHz by the time real work arrives.  The
        # warmup tile lives in the persistent pool: a scoped pool here would
        # hand its SBUF to the xq tiles, whose DMA would then wait for the
        # warmup matmuls to finish (WAR) — exactly the stall being avoided.
        wup = singles.tile([128, 128], BF16)
        nc.vector.memset(wup, 0.0)
        with tc.tile_pool(name="wupps", bufs=1, space="PSUM") as wup_ps:
            wps = wup_ps.tile([128, 128], F32)
            for _ in range(12):
                nc.tensor.matmul(wps, lhsT=(wup), rhs=(wup), start=True, stop=True)

        # ---- Phase A ----
        wfc_tiles = None
        with (
            tc.tile_pool(name="xt", bufs=5) as xt_pool,
            tc.tile_pool(name="wv", bufs=16) as wv_pool,
        ):
            # A1: Q^T, K^T (weight-stationary fp8 DoubleRow).  Wv^T tiles
            # stream on the scalar queue during K so they don't steal HBM
            # bandwidth from the startup-critical wq tiles.
            wv_tiles = []
            xts_v = None
            with (
                tc.tile_pool(name="wp", bufs=3) as w_pool,
                tc.tile_pool(name="pjps", bufs=2, space="PSUM") as pj_ps,
            ):
                for xi, (xT_dram, w_dram, OUT, scl) in enumerate((
                    (xq8, wq8, QT, 2.0**-12),
                    (xk8, wk8, KT, 2.0**-6),
                )):
                    xts = []
                    for q4 in range(4):
                        xq_t = xt_pool.tile([128, 8, T], FP8, tag="xt")
                        nc.sync.dma_start(
                            out=xq_t, in_=xT_dram[:, q4 * 8 : (q4 + 1) * 8, :]
                        )
                        xts.append(xq_t)
                    for pc in range(PC):
                        w = w_pool.tile([128, FT, 128], FP8, tag="w")
                        nc.gpsimd.dma_start(out=w, in_=w_dram[pc])
                        # Wv^T preload rides the same gpsimd DMA ring, behind
                        # wq/wk: ring FIFO + the w-pool's buffer pacing push
                        # the 8 MiB wv stream off the startup-critical path
                        # (engine program order does NOT order DMA dispatches,
                        # so a different queue would start transferring at t=0)
                        if len(wv_tiles) < NKP and (xi == 1 or pc >= 3):
                            wvt = wv_pool.tile([128, 2, P], FP8, tag="wv")
                            nc.gpsimd.dma_start(out=wvt, in_=wv8[len(wv_tiles)])
                            wv_tiles.append(wvt)
                        ps = pj_ps.tile([128, T], F32, tag="pj")
                        for jj in range(NKP):
                            m = (2 * jj) % 8
                            nc.tensor.matmul(
                                ps,
                                lhsT=(w[:, 2 * jj : 2 * jj + 2, :]),
                                rhs=(xts[jj // 4][:, m : m + 2, :]),
                                start=(jj == 0),
                                stop=(jj == NKP - 1),
                                perf_mode=DR,
                            )
                        nc.vector.tensor_scalar_mul(OUT[:, pc, :], ps, scl)

                # prefetch xv tiles while K finishes
                xts_v = []
                for q4 in range(4):
                    xv_t = xt_pool.tile([128, 8, T], FP8, tag="xt")
                    nc.sync.dma_start(
                        out=xv_t, in_=xv8[:, q4 * 8 : (q4 + 1) * 8, :]
                    )
                    xts_v.append(xv_t)

            # A2: V[t, p] (activation-stationary fp8 DoubleRow)
            with tc.tile_pool(name="vps", bufs=2, space="PSUM") as v_ps:
                for tcc in range(NTC):
                    vps = v_ps.tile([128, 4, 512], F32, tag="vps")
                    for kp in range(NKP):
                        m = (2 * kp) % 8
                        for pq in range(4):
                            nc.tensor.matmul(
                                vps[:, pq, :],
                                lhsT=(
                                    xts_v[kp // 4][:, m : m + 2, tcc * 128 : (tcc + 1) * 128]
                                ),
                                rhs=(wv_tiles[kp][:, :, pq * 512 : (pq + 1) * 512]),
                                start=(kp == 0),
                                stop=(kp == NKP - 1),
                                perf_mode=DR,
                            )
                    for pq in range(4):
                        # split psum->SBUF copies across DVE and ACT so the
                        # trailing copies (which gate phase B's psum banks)
                        # drain twice as fast
                        if pq % 2 == 0:
                            nc.vector.tensor_scalar_mul(
                                Vtp[:, tcc, pq * 512 : (pq + 1) * 512],
                                vps[:, pq, :],
                                2.0**-6,
                            )
                        else:
                            nc.scalar.mul(
                                Vtp[:, tcc, pq * 512 : (pq + 1) * 512],
                                vps[:, pq, :],
                                2.0**-6,
                            )

        # ---- Phase B: attention -> AT = 16*O^T [p, t] fp8 ----
        # fc weights preload on the idle sync queue (SBUF freed by phase A);
        # the phase-C residual tiles prefetch on gpsimd, also idle during B
        wfc_pool = ctx.enter_context(tc.tile_pool(name="wfc", bufs=8))
        wfc_tiles = []
        for j in range(NFC):
            wt = wfc_pool.tile([128, 2, F], FP8, tag="wfc")
            nc.sync.dma_start(out=wt, in_=wfc8[j])
            wfc_tiles.append(wt)
        xvr_pool = ctx.enter_context(tc.tile_pool(name="xvr", bufs=4))
        xv_tiles = []
        for tcc in range(NTC):
            xvt = xvr_pool.tile([128, F], FP16, tag="xv")
            nc.gpsimd.dma_start(out=xvt, in_=xv_res[tcc])
            xv_tiles.append(xvt)

        with (
            tc.tile_pool(name="asb", bufs=6) as asb,
            tc.tile_pool(name="stps", bufs=2, space="PSUM") as st_ps,
            tc.tile_pool(name="otps", bufs=1, space="PSUM") as ot_ps,
            tc.tile_pool(name="csps", bufs=1, space="PSUM") as cs_ps,
            tc.tile_pool(name="rbps", bufs=1, space="PSUM") as rb_ps,
        ):
            # two heads per iteration: wider ACT/PE ops, half the chain count
            iters = [(b, n) for b in range(BL) for n in range(0, NH, 2)]

            def emit_s(b, n):
                # S^T[d, (h2,c)] = sum_e K[d,e] Q[c,e] (head-transposed)
                t0 = b * C
                st = st_ps.tile([128, 2, 2, 256], F32, tag="st")
                for dc in range(2):
                    for h2 in range(2):
                        for et in range(2):
                            nc.tensor.matmul(
                                st[:, dc, h2, :],
                                lhsT=(
                                    KT[:, 2 * (n + h2) + et, t0 + dc * 128 : t0 + (dc + 1) * 128]
                                ),
                                rhs=(QT[:, 2 * (n + h2) + et, t0 : t0 + 256]),
                                start=(et == 0),
                                stop=(et == 1),
                            )
                return st

            st_next = emit_s(*iters[0])
            for it, (b, n) in enumerate(iters):
                    t0 = b * C
                    st = st_next
                    et_sb = asb.tile([128, 2, 2, 256], BF16, tag="et")
                    for dc in range(2):
                        nc.scalar.activation(
                            out=et_sb[:, dc], in_=st[:, dc], func=ACTF.Exp
                        )
                    # software pipeline: the next iteration's S matmuls issue
                    # before this iteration's cs/ot so the PE has independent
                    # work while the ACT exp chain runs (PE is in-order).
                    if it + 1 < len(iters):
                        st_next = emit_s(*iters[it + 1])
                    # column sums over d (partition axis) via (1/16)-matmul:
                    # cs = colsum/16, so r = exp(-ln(cs)) = 16/colsum and the
                    # fp8 A^T picks up the x16 fp8-friendly scale for free.
                    cs = cs_ps.tile([1, 512], F32, tag="cs")
                    for dc in range(2):
                        nc.tensor.matmul(
                            cs,
                            lhsT=(ones_col),
                            rhs=(et_sb[:, dc]),
                            start=(dc == 0),
                            stop=(dc == 1),
                        )
                    ln_sb = asb.tile([1, 512], F32, tag="lnsb")
                    nc.scalar.activation(out=ln_sb, in_=cs, func=ACTF.Ln)
                    r_sb = asb.tile([1, 512], BF16, tag="rsb")
                    nc.scalar.activation(out=r_sb, in_=ln_sb, func=ACTF.Exp, scale=-1.0)
                    # O^T[e, (h2,c)] = sum_d V[d,e] expS^T[d,c]  (V is [t,p])
                    ot = ot_ps.tile([128, 2, 2, 256], F32, tag="ot")
                    for h2 in range(2):
                        for ec in range(2):
                            for dc in range(2):
                                nc.tensor.matmul(
                                    ot[:, ec, h2, :],
                                    lhsT=(
                                        Vtp[
                                            :,
                                            b * 2 + dc,
                                            256 * (n + h2) + ec * 128 : 256 * (n + h2) + (ec + 1) * 128,
                                        ]
                                    ),
                                    rhs=(et_sb[:, dc, h2]),
                                    start=(dc == 0),
                                    stop=(dc == 1),
                                )
                    # broadcast r across partitions via K=1 matmul; issued
                    # after the ot matmuls so the ACT ln/exp chain hides
                    # behind PE work (the PE executes in program order)
                    rb = rb_ps.tile([128, 512], F32, tag="rb")
                    nc.tensor.matmul(rb, lhsT=(one_row), rhs=(r_sb), start=True, stop=True)
                    rb_sb = asb.tile([128, 512], F32, tag="rbs")
                    nc.vector.tensor_copy(out=rb_sb, in_=rb)
                    # AT p-chunks are laid out (ec, n) — heads adjacent — so
                    # both heads' chunks write in ONE strided DVE op per ec
                    # (the host permutes Wfc rows to match)
                    for ec in range(2):
                        nc.vector.tensor_mul(
                            out=AT[:, ec * 8 + n : ec * 8 + n + 2, t0 : t0 + 256],
                            in0=ot[:, ec],
                            in1=rb_sb,
                        )

        # ---- Phase C: FC + residual + BN partial stats (all x2048) ----
        with (
            tc.tile_pool(name="orow", bufs=2) as orow_pool,
            tc.tile_pool(name="sqp", bufs=2) as sq_pool,
            tc.tile_pool(name="fcps", bufs=8, space="PSUM") as fc_ps,
        ):
            for tcc in range(NTC):
                xvt = xv_tiles[tcc]
                orow = orow_pool.tile([128, F], FP16, tag="orow")
                for fc_ in range(NFC):
                    ps = fc_ps.tile([128, 512], F32, tag="fc")
                    for j in range(NFC):
                        nc.tensor.matmul(
                            ps,
                            lhsT=(AT[:, 2 * j : 2 * j + 2, tcc * 128 : (tcc + 1) * 128]),
                            rhs=(wfc_tiles[j][:, :, fc_ * 512 : (fc_ + 1) * 512]),
                            start=(j == 0),
                            stop=(j == NFC - 1),
                            perf_mode=DR,
                        )
                    ores = orow[:, fc_ * 512 : (fc_ + 1) * 512]
                    # residual: out = ps + 2048*xv (xv_res pre-scaled on host)
                    nc.vector.tensor_add(
                        ores, ps, xvt[:, fc_ * 512 : (fc_ + 1) * 512]
                    )
                    nc.vector.reduce_sum(
                        out=sums_buf[:, tcc, fc_ : fc_ + 1], in_=ores, axis=AX.X
                    )
                    sqt = sq_pool.tile([128, 512], F32, tag="sq")
                    nc.scalar.activation(
                        out=sqt,
                        in_=ores,
                        func=ACTF.Square,
                        accum_out=sqs_buf[:, tcc, fc_ : fc_ + 1],
                    )
                    if fc_ % 2 == 1:
                        qlo, qhi = (fc_ - 1) * 512, (fc_ + 1) * 512
                        nc.sync.dma_start(
                            out=out_blk[tcc][:, qlo:qhi], in_=orow[:, qlo:qhi]
                        )

            # fold partial sums: per t-chunk reduce over f-chunks, then add
            # the two batches per channel-half
            for tcc in range(NTC):
                nc.vector.reduce_sum(out=tmp4[:, tcc : tcc + 1], in_=sums_buf[:, tcc, :], axis=AX.X)
                nc.vector.reduce_sum(out=tmp4b[:, tcc : tcc + 1], in_=sqs_buf[:, tcc, :], axis=AX.X)
            nc.vector.tensor_add(stats_sb[:, 0:1], tmp4[:, 0:1], tmp4[:, 2:3])
            nc.vector.tensor_add(stats_sb[:, 1:2], tmp4[:, 1:2], tmp4[:, 3:4])
            nc.vector.tensor_add(stats_sb[:, 2:3], tmp4b[:, 0:1], tmp4b[:, 2:3])
            nc.vector.tensor_add(stats_sb[:, 3:4], tmp4b[:, 1:2], tmp4b[:, 3:4])
            nc.sync.dma_start(out=stats[:, :], in_=stats_sb)

    return nc


def build_k2() -> bass.Bass:
    nc = bass.Bass("TRN2", target_bir_lowering=False, debug=False, num_devices=NCORES)

    x_blk = nc.dram_tensor("x_blk", [NTC, 128, F], FP16, kind="ExternalInput")
    stats_all = nc.dram_tensor("stats_all", [4, 128, NCORES], F32, kind="ExternalInput")
    gamma2 = nc.dram_tensor("gamma2", [128, 2], F32, kind="ExternalInput")
    beta2 = nc.dram_tensor("beta2", [128, 2], F32, kind="ExternalInput")
    y_blk = nc.dram_tensor("y_blk", [NTC, 128, F], FP16, kind="ExternalOutput")

    with tile.TileContext(nc) as tc, ExitStack() as ctx:
        singles = ctx.enter_context(tc.tile_pool(name="singles", bufs=1))
        xpool = ctx.enter_context(tc.tile_pool(name="xin", bufs=4))
        ypool = ctx.enter_context(tc.tile_pool(name="yout", bufs=2))
        # x loads are the critical path: issue them first on the sync queue
        xin = []
        for tcc in range(NTC):
            t = xpool.tile([128, F], FP16, tag="in")
            nc.sync.dma_start(out=t, in_=x_blk[tcc])
            xin.append(t)
        # stats/constants on the gpsimd queue (host pre-transposed, so these
        # are few large-ish descriptors, not thousands of 4B packets)
        st_sb = singles.tile([128, 4, NCORES], F32)
        for j in range(4):
            nc.gpsimd.dma_start(out=st_sb[:, j, :], in_=stats_all[j])
        gam = singles.tile([128, 2], F32)
        nc.gpsimd.dma_start(out=gam, in_=gamma2[:, :])
        bet = singles.tile([128, 2], F32)
        nc.gpsimd.dma_start(out=bet, in_=beta2[:, :])
        eps_sb = singles.tile([128, 1], F32)
        nc.vector.memset(eps_sb, EPS)

        mean_sb = singles.tile([128, 2], F32)
        msq_sb = singles.tile([128, 2], F32)
        m2_sb = singles.tile([128, 2], F32)
        var_sb = singles.tile([128, 2], F32)
        std_sb = singles.tile([128, 2], F32)
        rstd_sb = singles.tile([128, 2], F32)
        scale_sb = singles.tile([128, 2], F32)
        shf_sb = singles.tile([128, 2], F32)
        tmp_sb = singles.tile([128, 2], F32)

        tot = singles.tile([128, 4], F32)
        nc.vector.reduce_sum(out=tot, in_=st_sb, axis=AX.X)
        inv_n = 1.0 / float(NTOT)
        nc.vector.tensor_scalar_mul(mean_sb, tot[:, 0:2], inv_n)
        nc.vector.tensor_scalar_mul(msq_sb, tot[:, 2:4], inv_n)
        nc.vector.tensor_mul(m2_sb, mean_sb, mean_sb)
        nc.vector.tensor_sub(var_sb, msq_sb, m2_sb)
        nc.scalar.activation(out=std_sb, in_=var_sb, func=ACTF.Sqrt, bias=eps_sb)
        nc.vector.reciprocal(out=rstd_sb, in_=std_sb)
        nc.vector.tensor_mul(scale_sb, gam, rstd_sb)
        nc.vector.tensor_mul(tmp_sb, mean_sb, scale_sb)
        nc.vector.tensor_sub(shf_sb, bet, tmp_sb)

        for tcc in range(NTC):
            j = tcc % 2
            y = ypool.tile([128, F], FP16, tag="y")
            nc.vector.tensor_scalar(
                out=y,
                in0=xin[tcc],
                scalar1=scale_sb[:, j : j + 1],
                scalar2=shf_sb[:, j : j + 1],
                op0=ALU.mult,
                op1=ALU.add,
            )
            nc.sync.dma_start(out=y_blk[tcc], in_=y)

    return nc


# ---------------------------------------------------------------------------
# Host-side layout prep
def _prep_weights(Wq, Wk, Wv, Wfc):
    import ml_dtypes

    f8 = ml_dtypes.float8_e4m3

    def blk_w(Wt):  # [P, F] -> [PC, 128, FT, 128] (f-major blocked, p-chunked)
        return np.ascontiguousarray(
            Wt.T.reshape(FT, 128, PC, 128).transpose(2, 1, 0, 3).astype(f8)
        )

    # x64 lifts fp8 values to ~unit std; undone in the PSUM->SBUF copies
    # (Q's 2^-12 copy scale = 1/64 fp8-undo * 1/TEMP softmax temperature)
    wq = blk_w(np.asarray(Wq, np.float32) * 64.0)
    wk = blk_w(np.asarray(Wk, np.float32) * 64.0)
    # Wv^T [F, P] -> [NKP, 128, 2, P] (k-tile pairs for DoubleRow rhs)
    wv = np.ascontiguousarray(
        (np.asarray(Wv, np.float32).T * 64.0)
        .reshape(NKP, 2, 128, P)
        .transpose(0, 2, 1, 3)
        .astype(f8)
    )
    # Wfc [F, P] -> Wfc^T [P, F], rows permuted from (n, ec) to (ec, n) order
    # to match AT's p-chunk layout, -> [NFC, 128, 2, F] (p-tile pairs), x128
    wfcT = (np.asarray(Wfc, np.float32).T * 128.0).reshape(NH, 2, 128, F)
    wfcT = wfcT.transpose(1, 0, 2, 3).reshape(P, F)
    wfc = np.ascontiguousarray(
        wfcT.reshape(NFC, 2, 128, F).transpose(0, 2, 1, 3).astype(f8)
    )
    return wq, wk, wv, wfc


_BUILT = {}


def _get_built(name):
    if name not in _BUILT:
        _BUILT[name] = build_k1() if name == "k1" else build_k2()
    return _BUILT[name]


def run_full(v, k, q, Wq, Wk, Wv, Wfc, gamma, beta, trace=False):
    """Returns (y [16,256,16,16,16] fp32, exec_ns_k1, exec_ns_k2)."""
    import ml_dtypes

    f8 = ml_dtypes.float8_e4m3
    if trace:
        _ensure_ntff_hook()
    q3 = np.asarray(q, np.float32).reshape(B, C, F)
    k3 = np.asarray(k, np.float32).reshape(B, C, F)
    v3 = np.asarray(v, np.float32).reshape(B, C, F)
    wq, wk, wv, wfc = _prep_weights(Wq, Wk, Wv, Wfc)

    def xpm(x):  # [T, F] -> partition-major x^T blocks [128, FT, T]
        return np.ascontiguousarray(x.T.reshape(FT, 128, T).transpose(1, 0, 2).astype(f8))

    in_maps = []
    for ci in range(NCORES):
        b0 = ci * BL
        xq = q3[b0 : b0 + BL].reshape(T, F)
        xk = k3[b0 : b0 + BL].reshape(T, F)
        xv = v3[b0 : b0 + BL].reshape(T, F)
        in_maps.append(
            {
                "xq8": xpm(xq),
                "xk8": xpm(xk),
                "xv8": xpm(xv),
                "wq8": wq,
                "wk8": wk,
                "wv8": wv,
                "wfc8": wfc,
                "xv_res": np.ascontiguousarray(
                    (xv * 2048.0).reshape(NTC, 128, F).astype(np.float16)
                ),
                "ones_c": np.full((128, 1), 1.0 / 16.0, ml_dtypes.bfloat16),
                "ones_r": np.ones((1, 128), ml_dtypes.bfloat16),
            }
        )

    nc1 = _get_built("k1")
    res1 = run_bass_kernel_spmd(nc1, in_maps, core_ids=list(range(NCORES)), trace=trace)
    t1 = res1.exec_time_ns

    # per-core stats come back [128, 4]; K2 wants [4, 128, NCORES]
    stats_all = np.ascontiguousarray(
        np.stack([res1.results[ci]["stats"] for ci in range(NCORES)]).transpose(2, 1, 0)
    )
    gamma2 = np.ascontiguousarray(np.asarray(gamma, np.float32).reshape(2, 128).T)
    beta2 = np.ascontiguousarray(np.asarray(beta, np.float32).reshape(2, 128).T)

    in_maps2 = [
        {
            "x_blk": res1.results[ci]["out_blk"],
            "stats_all": stats_all,
            "gamma2": gamma2,
            "beta2": beta2,
        }
        for ci in range(NCORES)
    ]
    nc2 = _get_built("k2")
    res2 = run_bass_kernel_spmd(nc2, in_maps2, core_ids=list(range(NCORES)), trace=trace)
    t2 = res2.exec_time_ns

    y = np.empty((B, C, F), np.float32)
    for ci in range(NCORES):
        y[ci * BL : (ci + 1) * BL] = (
            res2.results[ci]["y_blk"].astype(np.float32).reshape(T, F).reshape(BL, C, F)
        )
    return y.reshape(B, C, H, W, D), t1, t2


def kernel(**inputs) -> np.ndarray:
    y, _, _ = run_full(**inputs)
    return y


# revision 57
# speedup vs baseline: 1.0563x; 1.0210x over previous
"""Trainium2 Bass kernel for nn_MultiHeadAttention_71210557768100.

Data-parallel over batch: 16 batches -> 8 NeuronCores, 2 batches/core.
Per core, a single fused kernel (K1) computes QKV projections, per-head
softmax attention over channel tokens, the fc projection + residual, and
per-channel partial BatchNorm statistics. The host only concatenates the
8 cores' partial stats; a second tiny kernel (K2) reduces the stats
on-device and applies the BatchNorm scale/shift.

Perf strategy vs v1:
  * QKV projections and the fc matmul run in fp8(e4m3) with
    perf_mode=DoubleRow (2 k-tiles per pass, ~2x PE throughput).
    Weights are pre-scaled x64 (fc: x128) on the host so fp8 values sit
    in the normal range; scales are undone in the PSUM->SBUF copies.
  * V is projected activation-stationary producing V[t, p] directly, so
    the per-head PE transposes of V^T disappear from the attention phase.
  * fc output (+ residual added on the DVE from host-prescaled 2048*xv)
    is kept scaled by 2048 end-to-end: BatchNorm is scale-invariant, so
    K2's math is unchanged by the scaling.
  * All fc weights are preloaded into SBUF during the attention phase:
    phase C runs back-to-back matmuls with no DMA bubbles (keeps the PE
    HAM clock-gate warm).
  * Intermediate out and K2 I/O are fp16 with 1 MiB contiguous DMAs.

Layouts (per core; t = 512 local tokens, f = 4096, p = 2048):
  Q^T,K^T [p,t] <- lhsT=W8[f,p-chunk] (fp8 DR), rhs=x8^T[f,t] (fp8)
  V[t,p]       <- lhsT=x8^T[f,t-chunk] (fp8 DR), rhs=Wv8^T[f,p]
  S^T[d,c]     <- lhsT=K^T[e,d], rhs=Q^T[e,c]   (1/TEMP folded into Wq)
  softmax: exp (|S| small), colsum via (1/16)-matmul, r=16/sum via
           exp(-ln(cs)) LUTs, broadcast via K=1 matmul
  A^T=16*O^T [p,t] fp8 <- lhsT=V[d,e-chunk], rhs=exp(S^T); r folded
  FC[t,f]      <- lhsT=A^T[p,t-chunk] (fp8 DR), rhs=Wfc8^T[p,f]
                  + 2048*xv residual (DVE add, fp16)
"""

import os
import sys
import types
from contextlib import ExitStack

import numpy as np

if os.path.isdir("/opt/trn_rl_repo") and "/opt/trn_rl_repo" not in sys.path:
    sys.path.insert(0, "/opt/trn_rl_repo")

import concourse.bass as bass
import concourse.tile as tile
from concourse import mybir
from concourse.bass_utils import run_bass_kernel_spmd

F32 = mybir.dt.float32
F32R = mybir.dt.float32r
BF16 = mybir.dt.bfloat16
FP16 = mybir.dt.float16
FP8 = mybir.dt.float8e4
AX = mybir.AxisListType
ALU = mybir.AluOpType
ACTF = mybir.ActivationFunctionType
DR = mybir.MatmulPerfMode.DoubleRow

# Problem shapes (hardcoded per contract)
B, C, H, W, D = 16, 256, 16, 16, 16
F = H * W * D            # 4096 feature dim (in_pixels)
NH, LD = 8, 256
P = NH * LD              # 2048 projection dim
TEMP = float(np.sqrt(F))
EPS = 1e-5
NCORES = 8
BL = B // NCORES         # 2 local batches
T = BL * C               # 512 local tokens
FT = F // 128            # 32 f-tiles
PC = P // 128            # 16 p-chunks
NTC = T // 128           # 4 t-chunks
NFC = F // 512           # 8 f-chunks (512 wide)
NKP = FT // 2            # 16 f-tile pairs (DoubleRow)
NTOT = B * F             # BN reduction count per channel


# ---------------------------------------------------------------------------
# Workaround: this walrus build accepts at most one sync wait per Drain.
# TileContext's tail drain carries every outstanding sem wait on one SP
# drain; split them one wait per drain.
def _patch_tile_drain():
    import bass_rust as _br

    if getattr(tile.TileContext, "_drain_split_patched", False):
        return

    def _split_drain_and_barrier(self, tick_clock, wait_clock):
        nc = self.nc
        drain_inst = nc.sync.drain()
        wait_clock.add_sem_waits(
            drain_inst.ins, tile.ScopedClock({None: tick_clock.global_clock})
        )
        si = drain_inst.ins.sync_info
        waits = list(si.on_wait) if si is not None else []
        if len(waits) > 1:
            si.on_wait = waits[:1]
            for w in waits[1:]:
                d2 = nc.sync.drain()
                d2.ins.sync_info = _br.SyncInfo(on_wait=[w], on_update=[])
        nc.all_engine_barrier()
        assert self.sems is not None
        popped = nc._tile_sem_poison_stack.pop()
        assert popped is self._sem_poison
        nc.clear_and_free_semaphores(list(self.sems.allocated().values()))
        nc.all_engine_barrier()

    tile.TileContext._drain_and_barrier = _split_drain_and_barrier
    tile.TileContext._drain_split_patched = True


_patch_tile_drain()


# Workaround (general form): this walrus build accepts at most ONE sync
# wait per instruction. Post-process the serialized BIR: any instruction
# carrying N>1 waits keeps its last wait; the other N-1 move onto NoOp
# instructions inserted just before it on the same engine (same-engine
# program order makes this equivalent).
def _split_waits_in_json(raw: bytes) -> bytes:
    import json

    data = json.loads(raw)
    counter = [0]
    changed = False
    for fn in data.get("functions", []):
        for blk in fn.get("blocks", []):
            insts = blk.get("instructions", [])
            out = []
            for inst in insts:
                si = inst.get("sync_info")
                waits = si.get("on_wait") if si else None
                if waits and len(waits) > 1:
                    changed = True
                    eng = inst.get("engine")
                    for w in waits[:-1]:
                        counter[0] += 1
                        out.append(
                            {
                                "engine": eng,
                                "ins": [],
                                "name": f"I-wsplit-{counter[0]}",
                                "opcode": "NoOp",
                                "outs": [],
                                "sync_info": {"on_wait": [w], "on_update": []},
                            }
                        )
                    si["on_wait"] = waits[-1:]
                out.append(inst)
            if changed:
                blk["instructions"] = out
    if not changed:
        return raw
    return json.dumps(data).encode()


def _patch_wait_split():
    if getattr(bass.Bass, "_wait_split_patched", False):
        return
    orig = bass.Bass.to_json_bytes

    def to_json_bytes(self):
        return _split_waits_in_json(orig(self))

    bass.Bass.to_json_bytes = to_json_bytes
    bass.Bass._wait_split_patched = True


_patch_wait_split()


# NTFF profiling hook (for trace=True timing): register the ctypes-based
# hook if the antenv.axon_hooks module is missing in this image.
def _ensure_ntff_hook():
    try:
        import antenv.axon_hooks  # noqa: F401

        return
    except ImportError:
        pass
    try:
        from trn_agent_boot.trn_boot import _ntff_profile_via_ctypes

        hook = _ntff_profile_via_ctypes("/opt/axon/libaxon_pjrt.so")
    except Exception:
        hook = None
    mod = types.ModuleType("antenv.axon_hooks")
    mod.get_axon_ntff_profile_hook = lambda: hook
    mod.set_axon_ntff_profile_hook = lambda h: None
    sys.modules["antenv.axon_hooks"] = mod


# ---------------------------------------------------------------------------
def build_k1() -> bass.Bass:
    nc = bass.Bass("TRN2", target_bir_lowering=False, debug=False, num_devices=NCORES)

    xq8 = nc.dram_tensor("xq8", [128, FT, T], FP8, kind="ExternalInput")
    xk8 = nc.dram_tensor("xk8", [128, FT, T], FP8, kind="ExternalInput")
    xv8 = nc.dram_tensor("xv8", [128, FT, T], FP8, kind="ExternalInput")
    wq8 = nc.dram_tensor("wq8", [PC, 128, FT, 128], FP8, kind="ExternalInput")
    wk8 = nc.dram_tensor("wk8", [PC, 128, FT, 128], FP8, kind="ExternalInput")
    wv8 = nc.dram_tensor("wv8", [NKP, 128, 2, P], FP8, kind="ExternalInput")
    wfc8 = nc.dram_tensor("wfc8", [NFC, 128, 2, F], FP8, kind="ExternalInput")
    xv_res = nc.dram_tensor("xv_res", [NTC, 128, F], FP16, kind="ExternalInput")
    ones_c = nc.dram_tensor("ones_c", [128, 1], BF16, kind="ExternalInput")
    ones_r = nc.dram_tensor("ones_r", [1, 128], BF16, kind="ExternalInput")
    out_blk = nc.dram_tensor("out_blk", [NTC, 128, F], FP16, kind="ExternalOutput")
    stats = nc.dram_tensor("stats", [128, 4], F32, kind="ExternalOutput")

    with tile.TileContext(nc) as tc, ExitStack() as ctx:
        singles = ctx.enter_context(tc.tile_pool(name="singles", bufs=1))
        ones_col = singles.tile([128, 1], BF16)
        nc.sync.dma_start(out=ones_col, in_=ones_c[:, :])
        one_row = singles.tile([1, 128], BF16)
        nc.sync.dma_start(out=one_row, in_=ones_r[:, :])
        sums_buf = singles.tile([128, NTC, NFC], F32)
        sqs_buf = singles.tile([128, NTC, NFC], F32)
        tmp4 = singles.tile([128, 4], F32)
        tmp4b = singles.tile([128, 4], F32)
        stats_sb = singles.tile([128, 4], F32)

        qkv_pool = ctx.enter_context(tc.tile_pool(name="qkv", bufs=1))
        QT = qkv_pool.tile([128, PC, T], BF16, tag="QT")
        KT = qkv_pool.tile([128, PC, T], BF16, tag="KT")
        Vtp = qkv_pool.tile([128, NTC, P], BF16, tag="Vtp")

        at_pool = ctx.enter_context(tc.tile_pool(name="at", bufs=1))
        AT = at_pool.tile([128, PC, T], FP8)

        # PE warmup: dummy matmuls during the initial DMA wait so the HAM
        # clock-gate lifts to 2.4 GHz by the time real work arrives.  The
        # warmup tile lives in the persistent pool: a scoped pool here would
        # hand its SBUF to the xq tiles, whose DMA would then wait for the
        # warmup matmuls to finish (WAR) — exactly the stall being avoided.
        wup = singles.tile([128, 128], BF16)
        nc.vector.memset(wup, 0.0)
        with tc.tile_pool(name="wupps", bufs=1, space="PSUM") as wup_ps:
            wps = wup_ps.tile([128, 128], F32)
            for _ in range(12):
                nc.tensor.matmul(wps, lhsT=(wup), rhs=(wup), start=True, stop=True)

        # ---- Phase A ----
        wfc_tiles = None
        with (
            tc.tile_pool(name="xt", bufs=8) as xt_pool,
            tc.tile_pool(name="wv", bufs=16) as wv_pool,
        ):
            # A1: Q^T, K^T (weight-stationary fp8 DoubleRow).  Wv^T tiles
            # stream on the scalar queue during K so they don't steal HBM
            # bandwidth from the startup-critical wq tiles.
            wv_tiles = []
            xts_v = None
            with (
                tc.tile_pool(name="wp", bufs=3) as w_pool,
                tc.tile_pool(name="pjps", bufs=2, space="PSUM") as pj_ps,
            ):
                for xi, (xT_dram, w_dram, OUT, scl) in enumerate((
                    (xq8, wq8, QT, 2.0**-12),
                    (xk8, wk8, KT, 2.0**-6),
                )):
                    xts = []
                    for q4 in range(4):
                        xq_t = xt_pool.tile([128, 8, T], FP8, tag="xt")
                        nc.sync.dma_start(
                            out=xq_t, in_=xT_dram[:, q4 * 8 : (q4 + 1) * 8, :]
                        )
                        xts.append(xq_t)
                    for pc in range(PC):
                        w = w_pool.tile([128, FT, 128], FP8, tag="w")
                        nc.scalar.dma_start(out=w, in_=w_dram[pc])
                        # Wv^T preload rides the same gpsimd DMA ring, behind
                        # wq/wk: ring FIFO + the w-pool's buffer pacing push
                        # the 8 MiB wv stream off the startup-critical path
                        # (engine program order does NOT order DMA dispatches,
                        # so a different queue would start transferring at t=0)
                        if len(wv_tiles) < NKP and (xi == 1 or pc >= 3):
                            wvt = wv_pool.tile([128, 2, P], FP8, tag="wv")
                            nc.scalar.dma_start(out=wvt, in_=wv8[len(wv_tiles)])
                            wv_tiles.append(wvt)
                        ps = pj_ps.tile([128, T], F32, tag="pj")
                        for jj in range(NKP):
                            m = (2 * jj) % 8
                            nc.tensor.matmul(
                                ps,
                                lhsT=(w[:, 2 * jj : 2 * jj + 2, :]),
                                rhs=(xts[jj // 4][:, m : m + 2, :]),
                                start=(jj == 0),
                                stop=(jj == NKP - 1),
                                perf_mode=DR,
                            )
                        nc.vector.tensor_scalar_mul(OUT[:, pc, :], ps, scl)

                # prefetch xv tiles while K finishes
                xts_v = []
                for q4 in range(4):
                    xv_t = xt_pool.tile([128, 8, T], FP8, tag="xt")
                    nc.sync.dma_start(
                        out=xv_t, in_=xv8[:, q4 * 8 : (q4 + 1) * 8, :]
                    )
                    xts_v.append(xv_t)

            # A2: V[t, p] (activation-stationary fp8 DoubleRow)
            with tc.tile_pool(name="vps", bufs=2, space="PSUM") as v_ps:
                for tcc in range(NTC):
                    vps = v_ps.tile([128, 4, 512], F32, tag="vps")
                    for kp in range(NKP):
                        m = (2 * kp) % 8
                        for pq in range(4):
                            nc.tensor.matmul(
                                vps[:, pq, :],
                                lhsT=(
                                    xts_v[kp // 4][:, m : m + 2, tcc * 128 : (tcc + 1) * 128]
                                ),
                                rhs=(wv_tiles[kp][:, :, pq * 512 : (pq + 1) * 512]),
                                start=(kp == 0),
                                stop=(kp == NKP - 1),
                                perf_mode=DR,
                            )
                    for pq in range(4):
                        # split psum->SBUF copies across DVE and ACT so the
                        # trailing copies (which gate phase B's psum banks)
                        # drain twice as fast
                        if pq % 2 == 0:
                            nc.vector.tensor_scalar_mul(
                                Vtp[:, tcc, pq * 512 : (pq + 1) * 512],
                                vps[:, pq, :],
                                2.0**-6,
                            )
                        else:
                            nc.scalar.mul(
                                Vtp[:, tcc, pq * 512 : (pq + 1) * 512],
                                vps[:, pq, :],
                                2.0**-6,
                            )

        # ---- Phase B: attention -> AT = 16*O^T [p, t] fp8 ----
        # fc weights preload on the idle sync queue (SBUF freed by phase A);
        # the phase-C residual tiles prefetch on gpsimd, also idle during B
        wfc_pool = ctx.enter_context(tc.tile_pool(name="wfc", bufs=8))
        wfc_tiles = []
        for j in range(NFC):
            wt = wfc_pool.tile([128, 2, F], FP8, tag="wfc")
            nc.sync.dma_start(out=wt, in_=wfc8[j])
            wfc_tiles.append(wt)
        xvr_pool = ctx.enter_context(tc.tile_pool(name="xvr", bufs=4))
        xv_tiles = []
        for tcc in range(NTC):
            xvt = xvr_pool.tile([128, F], FP16, tag="xv")
            nc.sync.dma_start(out=xvt, in_=xv_res[tcc])
            xv_tiles.append(xvt)

        with (
            tc.tile_pool(name="asb", bufs=6) as asb,
            tc.tile_pool(name="stps", bufs=2, space="PSUM") as st_ps,
            tc.tile_pool(name="otps", bufs=1, space="PSUM") as ot_ps,
            tc.tile_pool(name="csps", bufs=1, space="PSUM") as cs_ps,
            tc.tile_pool(name="rbps", bufs=1, space="PSUM") as rb_ps,
        ):
            # two heads per iteration: wider ACT/PE ops, half the chain count
            iters = [(b, n) for b in range(BL) for n in range(0, NH, 2)]

            def emit_s(b, n):
                # S^T[d, (h2,c)] = sum_e K[d,e] Q[c,e] (head-transposed)
                t0 = b * C
                st = st_ps.tile([128, 2, 2, 256], F32, tag="st")
                for dc in range(2):
                    for h2 in range(2):
                        for et in range(2):
                            nc.tensor.matmul(
                                st[:, dc, h2, :],
                                lhsT=(
                                    KT[:, 2 * (n + h2) + et, t0 + dc * 128 : t0 + (dc + 1) * 128]
                                ),
                                rhs=(QT[:, 2 * (n + h2) + et, t0 : t0 + 256]),
                                start=(et == 0),
                                stop=(et == 1),
                            )
                return st

            st_next = emit_s(*iters[0])
            for it, (b, n) in enumerate(iters):
                    t0 = b * C
                    st = st_next
                    et_sb = asb.tile([128, 2, 2, 256], BF16, tag="et")
                    for dc in range(2):
                        nc.scalar.activation(
                            out=et_sb[:, dc], in_=st[:, dc], func=ACTF.Exp
                        )
                    # software pipeline: the next iteration's S matmuls issue
                    # before this iteration's cs/ot so the PE has independent
                    # work while the ACT exp chain runs (PE is in-order).
                    if it + 1 < len(iters):
                        st_next = emit_s(*iters[it + 1])
                    # column sums over d (partition axis) via (1/16)-matmul:
                    # cs = colsum/16, so r = exp(-ln(cs)) = 16/colsum and the
                    # fp8 A^T picks up the x16 fp8-friendly scale for free.
                    cs = cs_ps.tile([1, 512], F32, tag="cs")
                    for dc in range(2):
                        nc.tensor.matmul(
                            cs,
                            lhsT=(ones_col),
                            rhs=(et_sb[:, dc]),
                            start=(dc == 0),
                            stop=(dc == 1),
                        )
                    ln_sb = asb.tile([1, 512], F32, tag="lnsb")
                    nc.scalar.activation(out=ln_sb, in_=cs, func=ACTF.Ln)
                    r_sb = asb.tile([1, 512], BF16, tag="rsb")
                    nc.scalar.activation(out=r_sb, in_=ln_sb, func=ACTF.Exp, scale=-1.0)
                    # O^T[e, (h2,c)] = sum_d V[d,e] expS^T[d,c]  (V is [t,p])
                    ot = ot_ps.tile([128, 2, 2, 256], F32, tag="ot")
                    for h2 in range(2):
                        for ec in range(2):
                            for dc in range(2):
                                nc.tensor.matmul(
                                    ot[:, ec, h2, :],
                                    lhsT=(
                                        Vtp[
                                            :,
                                            b * 2 + dc,
                                            256 * (n + h2) + ec * 128 : 256 * (n + h2) + (ec + 1) * 128,
                                        ]
                                    ),
                                    rhs=(et_sb[:, dc, h2]),
                                    start=(dc == 0),
                                    stop=(dc == 1),
                                )
                    # broadcast r across partitions via K=1 matmul; issued
                    # after the ot matmuls so the ACT ln/exp chain hides
                    # behind PE work (the PE executes in program order)
                    rb = rb_ps.tile([128, 512], F32, tag="rb")
                    nc.tensor.matmul(rb, lhsT=(one_row), rhs=(r_sb), start=True, stop=True)
                    rb_sb = asb.tile([128, 512], F32, tag="rbs")
                    nc.vector.tensor_copy(out=rb_sb, in_=rb)
                    # AT p-chunks are laid out (ec, n) — heads adjacent — so
                    # both heads' chunks write in ONE strided DVE op per ec
                    # (the host permutes Wfc rows to match)
                    for ec in range(2):
                        nc.vector.tensor_mul(
                            out=AT[:, ec * 8 + n : ec * 8 + n + 2, t0 : t0 + 256],
                            in0=ot[:, ec],
                            in1=rb_sb,
                        )

        # ---- Phase C: FC + residual + BN partial stats (all x2048) ----
        with (
            tc.tile_pool(name="orow", bufs=2) as orow_pool,
            tc.tile_pool(name="sqp", bufs=2) as sq_pool,
            tc.tile_pool(name="fcps", bufs=8, space="PSUM") as fc_ps,
        ):
            for tcc in range(NTC):
                xvt = xv_tiles[tcc]
                orow = orow_pool.tile([128, F], FP16, tag="orow")
                for fc_ in range(NFC):
                    ps = fc_ps.tile([128, 512], F32, tag="fc")
                    for j in range(NFC):
                        nc.tensor.matmul(
                            ps,
                            lhsT=(AT[:, 2 * j : 2 * j + 2, tcc * 128 : (tcc + 1) * 128]),
                            rhs=(wfc_tiles[j][:, :, fc_ * 512 : (fc_ + 1) * 512]),
                            start=(j == 0),
                            stop=(j == NFC - 1),
                            perf_mode=DR,
                        )
                    ores = orow[:, fc_ * 512 : (fc_ + 1) * 512]
                    # residual: out = ps + 2048*xv (xv_res pre-scaled on host)
                    nc.vector.tensor_add(
                        ores, ps, xvt[:, fc_ * 512 : (fc_ + 1) * 512]
                    )
                    nc.vector.reduce_sum(
                        out=sums_buf[:, tcc, fc_ : fc_ + 1], in_=ores, axis=AX.X
                    )
                    sqt = sq_pool.tile([128, 512], F32, tag="sq")
                    nc.scalar.activation(
                        out=sqt,
                        in_=ores,
                        func=ACTF.Square,
                        accum_out=sqs_buf[:, tcc, fc_ : fc_ + 1],
                    )
                    if fc_ % 2 == 1:
                        qlo, qhi = (fc_ - 1) * 512, (fc_ + 1) * 512
                        nc.sync.dma_start(
                            out=out_blk[tcc][:, qlo:qhi], in_=orow[:, qlo:qhi]
                        )

            # fold partial sums: per t-chunk reduce over f-chunks, then add
            # the two batches per channel-half
            for tcc in range(NTC):
                nc.vector.reduce_sum(out=tmp4[:, tcc : tcc + 1], in_=sums_buf[:, tcc, :], axis=AX.X)
                nc.vector.reduce_sum(out=tmp4b[:, tcc : tcc + 1], in_=sqs_buf[:, tcc, :], axis=AX.X)
            nc.vector.tensor_add(stats_sb[:, 0:1], tmp4[:, 0:1], tmp4[:, 2:3])
            nc.vector.tensor_add(stats_sb[:, 1:2], tmp4[:, 1:2], tmp4[:, 3:4])
            nc.vector.tensor_add(stats_sb[:, 2:3], tmp4b[:, 0:1], tmp4b[:, 2:3])
            nc.vector.tensor_add(stats_sb[:, 3:4], tmp4b[:, 1:2], tmp4b[:, 3:4])
            nc.sync.dma_start(out=stats[:, :], in_=stats_sb)

    return nc


def build_k2() -> bass.Bass:
    nc = bass.Bass("TRN2", target_bir_lowering=False, debug=False, num_devices=NCORES)

    x_blk = nc.dram_tensor("x_blk", [NTC, 128, F], FP16, kind="ExternalInput")
    stats_all = nc.dram_tensor("stats_all", [4, 128, NCORES], F32, kind="ExternalInput")
    gamma2 = nc.dram_tensor("gamma2", [128, 2], F32, kind="ExternalInput")
    beta2 = nc.dram_tensor("beta2", [128, 2], F32, kind="ExternalInput")
    y_blk = nc.dram_tensor("y_blk", [NTC, 128, F], FP16, kind="ExternalOutput")

    with tile.TileContext(nc) as tc, ExitStack() as ctx:
        singles = ctx.enter_context(tc.tile_pool(name="singles", bufs=1))
        xpool = ctx.enter_context(tc.tile_pool(name="xin", bufs=4))
        ypool = ctx.enter_context(tc.tile_pool(name="yout", bufs=2))
        # x loads are the critical path: issue them first on the sync queue
        xin = []
        for tcc in range(NTC):
            t = xpool.tile([128, F], FP16, tag="in")
            nc.sync.dma_start(out=t, in_=x_blk[tcc])
            xin.append(t)
        # stats/constants on the gpsimd queue (host pre-transposed, so these
        # are few large-ish descriptors, not thousands of 4B packets)
        st_sb = singles.tile([128, 4, NCORES], F32)
        for j in range(4):
            nc.gpsimd.dma_start(out=st_sb[:, j, :], in_=stats_all[j])
        gam = singles.tile([128, 2], F32)
        nc.gpsimd.dma_start(out=gam, in_=gamma2[:, :])
        bet = singles.tile([128, 2], F32)
        nc.gpsimd.dma_start(out=bet, in_=beta2[:, :])
        eps_sb = singles.tile([128, 1], F32)
        nc.vector.memset(eps_sb, EPS)

        mean_sb = singles.tile([128, 2], F32)
        msq_sb = singles.tile([128, 2], F32)
        m2_sb = singles.tile([128, 2], F32)
        var_sb = singles.tile([128, 2], F32)
        std_sb = singles.tile([128, 2], F32)
        rstd_sb = singles.tile([128, 2], F32)
        scale_sb = singles.tile([128, 2], F32)
        shf_sb = singles.tile([128, 2], F32)
        tmp_sb = singles.tile([128, 2], F32)

        tot = singles.tile([128, 4], F32)
        nc.vector.reduce_sum(out=tot, in_=st_sb, axis=AX.X)
        inv_n = 1.0 / float(NTOT)
        nc.vector.tensor_scalar_mul(mean_sb, tot[:, 0:2], inv_n)
        nc.vector.tensor_scalar_mul(msq_sb, tot[:, 2:4], inv_n)
        nc.vector.tensor_mul(m2_sb, mean_sb, mean_sb)
        nc.vector.tensor_sub(var_sb, msq_sb, m2_sb)
        nc.scalar.activation(out=std_sb, in_=var_sb, func=ACTF.Sqrt, bias=eps_sb)
        nc.vector.reciprocal(out=rstd_sb, in_=std_sb)
        nc.vector.tensor_mul(scale_sb, gam, rstd_sb)
        nc.vector.tensor_mul(tmp_sb, mean_sb, scale_sb)
        nc.vector.tensor_sub(shf_sb, bet, tmp_sb)

        for tcc in range(NTC):
            j = tcc % 2
            y = ypool.tile([128, F], FP16, tag="y")
            nc.vector.tensor_scalar(
                out=y,
                in0=xin[tcc],
                scalar1=scale_sb[:, j : j + 1],
                scalar2=shf_sb[:, j : j + 1],
                op0=ALU.mult,
                op1=ALU.add,
            )
            nc.sync.dma_start(out=y_blk[tcc], in_=y)

    return nc


# ---------------------------------------------------------------------------
# Host-side layout prep
def _prep_weights(Wq, Wk, Wv, Wfc):
    import ml_dtypes

    f8 = ml_dtypes.float8_e4m3

    def blk_w(Wt):  # [P, F] -> [PC, 128, FT, 128] (f-major blocked, p-chunked)
        return np.ascontiguousarray(
            Wt.T.reshape(FT, 128, PC, 128).transpose(2, 1, 0, 3).astype(f8)
        )

    # x64 lifts fp8 values to ~unit std; undone in the PSUM->SBUF copies
    # (Q's 2^-12 copy scale = 1/64 fp8-undo * 1/TEMP softmax temperature)
    wq = blk_w(np.asarray(Wq, np.float32) * 64.0)
    wk = blk_w(np.asarray(Wk, np.float32) * 64.0)
    # Wv^T [F, P] -> [NKP, 128, 2, P] (k-tile pairs for DoubleRow rhs)
    wv = np.ascontiguousarray(
        (np.asarray(Wv, np.float32).T * 64.0)
        .reshape(NKP, 2, 128, P)
        .transpose(0, 2, 1, 3)
        .astype(f8)
    )
    # Wfc [F, P] -> Wfc^T [P, F], rows permuted from (n, ec) to (ec, n) order
    # to match AT's p-chunk layout, -> [NFC, 128, 2, F] (p-tile pairs), x128
    wfcT = (np.asarray(Wfc, np.float32).T * 128.0).reshape(NH, 2, 128, F)
    wfcT = wfcT.transpose(1, 0, 2, 3).reshape(P, F)
    wfc = np.ascontiguousarray(
        wfcT.reshape(NFC, 2, 128, F).transpose(0, 2, 1, 3).astype(f8)
    )
    return wq, wk, wv, wfc


_BUILT = {}


def _get_built(name):
    if name not in _BUILT:
        _BUILT[name] = build_k1() if name == "k1" else build_k2()
    return _BUILT[name]


def run_full(v, k, q, Wq, Wk, Wv, Wfc, gamma, beta, trace=False):
    """Returns (y [16,256,16,16,16] fp32, exec_ns_k1, exec_ns_k2)."""
    import ml_dtypes

    f8 = ml_dtypes.float8_e4m3
    if trace:
        _ensure_ntff_hook()
    q3 = np.asarray(q, np.float32).reshape(B, C, F)
    k3 = np.asarray(k, np.float32).reshape(B, C, F)
    v3 = np.asarray(v, np.float32).reshape(B, C, F)
    wq, wk, wv, wfc = _prep_weights(Wq, Wk, Wv, Wfc)

    def xpm(x):  # [T, F] -> partition-major x^T blocks [128, FT, T]
        return np.ascontiguousarray(x.T.reshape(FT, 128, T).transpose(1, 0, 2).astype(f8))

    in_maps = []
    for ci in range(NCORES):
        b0 = ci * BL
        xq = q3[b0 : b0 + BL].reshape(T, F)
        xk = k3[b0 : b0 + BL].reshape(T, F)
        xv = v3[b0 : b0 + BL].reshape(T, F)
        in_maps.append(
            {
                "xq8": xpm(xq),
                "xk8": xpm(xk),
                "xv8": xpm(xv),
                "wq8": wq,
                "wk8": wk,
                "wv8": wv,
                "wfc8": wfc,
                "xv_res": np.ascontiguousarray(
                    (xv * 2048.0).reshape(NTC, 128, F).astype(np.float16)
                ),
                "ones_c": np.full((128, 1), 1.0 / 16.0, ml_dtypes.bfloat16),
                "ones_r": np.ones((1, 128), ml_dtypes.bfloat16),
            }
        )

    nc1 = _get_built("k1")
    res1 = run_bass_kernel_spmd(nc1, in_maps, core_ids=list(range(NCORES)), trace=trace)
    t1 = res1.exec_time_ns

    # per-core stats come back [128, 4]; K2 wants [4, 128, NCORES]
    stats_all = np.ascontiguousarray(
        np.stack([res1.results[ci]["stats"] for ci in range(NCORES)]).transpose(2, 1, 0)
    )
    gamma2 = np.ascontiguousarray(np.asarray(gamma, np.float32).reshape(2, 128).T)
    beta2 = np.ascontiguousarray(np.asarray(beta, np.float32).reshape(2, 128).T)

    in_maps2 = [
        {
            "x_blk": res1.results[ci]["out_blk"],
            "stats_all": stats_all,
            "gamma2": gamma2,
            "beta2": beta2,
        }
        for ci in range(NCORES)
    ]
    nc2 = _get_built("k2")
    res2 = run_bass_kernel_spmd(nc2, in_maps2, core_ids=list(range(NCORES)), trace=trace)
    t2 = res2.exec_time_ns

    y = np.empty((B, C, F), np.float32)
    for ci in range(NCORES):
        y[ci * BL : (ci + 1) * BL] = (
            res2.results[ci]["y_blk"].astype(np.float32).reshape(T, F).reshape(BL, C, F)
        )
    return y.reshape(B, C, H, W, D), t1, t2


def kernel(**inputs) -> np.ndarray:
    y, _, _ = run_full(**inputs)
    return y
